# revision 17
# baseline (speedup 1.0000x reference)
"""Masked attention kernel for Trainium2, data-parallel over 8 NeuronCores.

Problem: out[q,b,:] = softmax-ish(LN(query Wq^T+bq) @ LN(key Wk^T+bk)^T / sqrt(H),
masked by query_mask & key_mask, with the reference's idiosyncratic
exp(s - 2*rowmax) / (sum + 0.001) normalization) @ value.

Key observations exploited:
 - The reference fills masked scores with the GLOBAL min before the row max.
   Every unmasked score >= global min, so the row max equals the max over
   unmasked entries whenever one exists; fully-masked rows output exactly 0.
   Hence zero cross-batch communication: B=8 batches map 1:1 onto 8 cores.
 - Masked-out query rows produce zero output rows; masked-out keys contribute
   nothing.  Both masks are ~50% dense, so each core computes attention only
   over compacted (host-gathered) rows, padded to a fixed size.
 - exp(s - 2m)/(sum + 0.001) == exp(s)/(sum' + 0.001*exp(2m)), and
   exp(2m) == (rowmax of exp(s))^2.  So we take exp with no shift at all
   (scaled scores are O(5), safely in range) and correct the denominator with
   0.001*emax^2 - npad (each padded key column contributes exactly exp(0)=1).
   This kills the entire PSUM row-max reduction pass.
 - All layout transposes (projection -> [h,seq] operands, exp(S) -> [k,q]
   stationaries for the PV matmul) run on the DMA engines' XBAR transpose
   path instead of the PE array, leaving the PE a pure matmul stream.
 - One PSUM pool with per-tag slots: 5 rotating projection/PV-accumulator
   banks plus one bank per score chunk, so phase B starts with zero
   pool-scope WAR against phase A and per-chunk exps release score banks
   incrementally.

Host side: compact/pad/transpose per batch (cheap numpy), run the SPMD NEFF,
scatter results back into the full [Q,B,H] output.
"""

import numpy as np
import ml_dtypes

import concourse.bacc as bacc
import concourse.bass as bass
import concourse.tile as tile
from concourse import mybir
from concourse.bass_utils import run_bass_kernel_spmd


def _ensure_axon_hooks():
    """concourse's trace path imports antenv.axon_hooks, which is absent in
    some containers; provide a no-op stand-in so BASS_TRACE=1 degrades to
    untraced execution instead of crashing."""
    try:
        import antenv.axon_hooks  # noqa: F401
    except ImportError:
        import sys as _sys
        import types as _types
        m = _types.ModuleType("antenv.axon_hooks")
        m._h = None
        m.set_axon_ntff_profile_hook = lambda h: setattr(m, "_h", h)
        m.get_axon_ntff_profile_hook = lambda: m._h
        _sys.modules["antenv.axon_hooks"] = m


_ensure_axon_hooks()

F32 = mybir.dt.float32
BF16 = mybir.dt.bfloat16
AX = mybir.AxisListType.X
AF = mybir.ActivationFunctionType
ALU = mybir.AluOpType

H = 512
HC = H // 128          # contraction chunks over the hidden dim
NCORES = 8
RSQRT_H = 1.0 / float(np.sqrt(np.float32(H)))
EPS = 1e-5

_cache = {}
last_results = None


def _tile_groups(nt):
    """Partition nt 128-wide key tiles into the fewest PSUM-bank groups of
    <=4 tiles (one matmul chunk / PSUM bank per group)."""
    ng = -(-nt // 4)
    base, rem = divmod(nt, ng)
    return [base + (1 if i < rem else 0) for i in range(ng)]


def _build(pad, biasq, biask, affq, affk):
    nt = pad // 128
    groups = _tile_groups(nt)
    ng = len(groups)
    nquads = -(-nt // 4)

    nc = bacc.Bacc(None, target_bir_lowering=False, debug=False, enable_asserts=False,
                   enable_partition_id=False)

    xqT_d = nc.declare_dram_parameter("xqT", [nt, 128, HC, 128], BF16, isOutput=False)
    xkT_d = nc.declare_dram_parameter("xkT", [nt, 128, HC, 128], BF16, isOutput=False)
    v_d = nc.declare_dram_parameter("v", [pad, H], BF16, isOutput=False)
    npad_d = nc.declare_dram_parameter("npad", [1, 1], F32, isOutput=False)
    km_d = None
    if biask or affk:
        km_d = nc.declare_dram_parameter("km01", [pad, 1], F32, isOutput=False)
    wqT_d = nc.declare_dram_parameter("WqT", [H, H], BF16, isOutput=False)
    wkT_d = nc.declare_dram_parameter("WkT", [H, H], BF16, isOutput=False)
    extras_d = {}
    if biasq:
        extras_d["bq"] = nc.declare_dram_parameter("bq", [1, H], F32, isOutput=False)
    if biask:
        extras_d["bk"] = nc.declare_dram_parameter("bk", [1, H], F32, isOutput=False)
    if affq:
        extras_d["gq"] = nc.declare_dram_parameter("gq", [1, H], F32, isOutput=False)
        extras_d["betaq"] = nc.declare_dram_parameter("betaq", [1, H], F32, isOutput=False)
    if affk:
        extras_d["gk"] = nc.declare_dram_parameter("gk", [1, H], F32, isOutput=False)
        extras_d["betak"] = nc.declare_dram_parameter("betak", [1, H], F32, isOutput=False)
    out_d = nc.declare_dram_parameter("out", [pad, H], BF16, isOutput=True)

    with tile.TileContext(nc) as tc:
        with (
            tc.tile_pool(name="persist", bufs=1) as persist,
            tc.tile_pool(name="small", bufs=10) as small,
            tc.tile_pool(name="lnt", bufs=4) as lnt,
            tc.tile_pool(name="ework", bufs=4) as ework,
            tc.tile_pool(name="osb", bufs=3) as osbp,
            tc.tile_pool(name="ps", bufs=1, space="PSUM") as ps,
        ):
            eps_t = persist.tile([128, 1], F32)
            nc.vector.memset(eps_t[:], EPS)

            # K-side inputs on the sync queue (its descriptors drain first),
            # Q-side on scalar; phase A runs all K jobs before Q jobs so the
            # last K transpose hides under the Q projections.  Inputs and the
            # transposed Q operand live in per-pair tiles so consumers wait on
            # exactly one DMA (tile-granular dependency tracking).
            wk_sb = persist.tile([128, HC, H], BF16)
            wq_sb = persist.tile([128, HC, H], BF16)
            wk_r = wkT_d[:, :].rearrange("(c p) i -> p c i", p=128)
            wq_r = wqT_d[:, :].rearrange("(c p) i -> p c i", p=128)
            xkp = [persist.tile([128, 2, HC, 128], BF16, tag=f"xkp{j}", name=f"xkp{j}")
                   for j in range(-(-nt // 2))]
            xqp = [persist.tile([128, 2, HC, 128], BF16, tag=f"xqp{j}", name=f"xqp{j}")
                   for j in range(-(-nt // 2))]
            nc.sync.dma_start(out=wk_sb[:], in_=wk_r[:])
            for j in range(-(-nt // 2)):
                t0, t1 = 2 * j, min(2 * j + 2, nt)
                nc.sync.dma_start(out=xkp[j][:, 0:t1 - t0, :, :],
                                  in_=xkT_d[t0:t1, :, :, :].rearrange("t p c u -> p t c u"))
            nc.scalar.dma_start(out=wq_sb[:], in_=wq_r[:])
            for j in range(-(-nt // 2)):
                t0, t1 = 2 * j, min(2 * j + 2, nt)
                nc.scalar.dma_start(out=xqp[j][:, 0:t1 - t0, :, :],
                                    in_=xqT_d[t0:t1, :, :, :].rearrange("t p c u -> p t c u"))
            npad_t = persist.tile([128, 1], F32)
            kmA_sb = None
            if km_d is not None:
                kmA_sb = persist.tile([128, nt], F32)
                nc.gpsimd.dma_start(out=kmA_sb[:], in_=km_d[:, :].rearrange("(n p) o -> p (n o)", p=128))
            bcast = {}
            for name, dram in extras_d.items():
                tb = persist.tile([128, H], F32, tag=f"bc_{name}")
                src = dram[:, :]
                src = bass.AP(tensor=src.tensor, offset=src.offset, ap=[[0, 128]] + [src.ap[-1]])
                nc.gpsimd.dma_start(out=tb[:], in_=src)
                bcast[name] = tb

            # [128 h_sub, tile, h_chunk, 128 seq] operand layouts for attention.
            # kT is one tile (QK^T moving operand spans key tiles; every use
            # needs all of it anyway); qT is per-pair (each QK^T stationary
            # needs exactly one pair's XBAR transpose).
            qTq = [persist.tile([128, 4, HC, 128], BF16, tag=f"qTq{j}", name=f"qTq{j}")
                   for j in range(nquads)]
            kT_sb = persist.tile([128, nt, HC, 128], BF16)
            v_sb = persist.tile([128, nt, H], BF16)

            # ---- phase A: project + layernorm (XBAR-transposed evacuation) ------
            jobs = [(1, t) for t in range(nt)] + [(0, t) for t in range(nt)]
            ps_of = {}
            lnp_of = {}

            def proj(i):
                s, t = jobs[i]
                x_sb = (xqp, xkp)[s][t // 2][:, t % 2, :, :]
                w_sb = (wq_sb, wk_sb)[s]
                p = ps.tile([128, H], F32, tag="u", bufs=8)
                ps_of[i] = p
                for c in range(HC):
                    nc.tensor.matmul(p[:], x_sb[:, c, :],
                                     w_sb[:, c, :], start=(c == 0), stop=(c == HC - 1))

            def ln_and_tp(i):
                s, t = jobs[i]
                use_bias = (biasq, biask)[s]
                use_aff = (affq, affk)[s]
                p = ps_of.pop(i)
                if use_bias:
                    nc.vector.tensor_add(p[:], p[:], bcast[("bq", "bk")[s]][:])
                stats = small.tile([128, 6], F32, tag="stats")
                nc.vector.bn_stats(out=stats[:], in_=p[:])
                mv = small.tile([128, 2], F32, tag="mv")
                nc.vector.bn_aggr(out=mv[:], in_=stats[:])
                sd = small.tile([128, 1], F32, tag="sd")
                nc.scalar.activation(out=sd[:], in_=mv[:, 1:2], func=AF.Sqrt,
                                     bias=eps_t[:], scale=1.0)
                rstd = small.tile([128, 1], F32, tag="rstd")
                nc.vector.reciprocal(out=rstd[:], in_=sd[:])
                nmr = small.tile([128, 1], F32, tag="nmr")
                nc.gpsimd.tensor_mul(nmr[:], mv[:, 0:1], rstd[:])
                nc.gpsimd.tensor_scalar_mul(nmr[:], nmr[:], -1.0)
                # batch up to four tiles of one side into a single XBAR
                # transpose (amortizes the ~1.3us per-transpose engine cost)
                pid, slot = t // 4, t % 4
                if slot == 0:
                    lnp = lnt.tile([128, 4, H], BF16, tag="lnp")
                    lnp_of[(s, pid)] = lnp
                else:
                    lnp = lnp_of[(s, pid)]
                ln = lnp[:, slot, :]
                if use_aff:
                    ln32 = lnt.tile([128, H], F32, tag="ln32")
                    nc.scalar.activation(out=ln32[:], in_=p[:], func=AF.Identity,
                                         bias=nmr[:], scale=rstd[:])
                    nc.vector.tensor_mul(ln32[:], ln32[:], bcast[("gq", "gk")[s]][:])
                    nc.vector.tensor_add(ln, ln32[:], bcast[("betaq", "betak")[s]][:])
                else:
                    nc.scalar.activation(out=ln, in_=p[:], func=AF.Identity,
                                         bias=nmr[:], scale=rstd[:])
                if s == 1 and kmA_sb is not None:
                    nc.vector.tensor_scalar_mul(ln, ln, kmA_sb[:, t:t + 1])
                # XBAR transpose on sync only: dst[p, c, q] = ln[q, c*128+p]
                if slot == 3 or t == nt - 1:
                    if s == 1:
                        dst = kT_sb[:, 4 * pid:4 * pid + slot + 1, :, :]
                    else:
                        dst = qTq[pid][:, 0:slot + 1, :, :]
                    if slot == 3:
                        nc.sync.dma_start_transpose(out=dst, in_=lnp[:])
                    else:
                        nc.sync.dma_start_transpose(out=dst, in_=lnp[:, 0:slot + 1, :])
                    lnp_of.pop((s, pid))

            DEPTH = 7
            for i in range(min(DEPTH, len(jobs))):
                proj(i)
            for i in range(len(jobs)):
                if i + DEPTH < len(jobs):
                    proj(i + DEPTH)
                ln_and_tp(i)
                if i == 6:
                    # defer bulk/broadcast loads so their descriptors don't
                    # starve the phase-A input stream
                    np_src = npad_d[:, :]
                    np_src = bass.AP(tensor=np_src.tensor, offset=np_src.offset,
                                     ap=[[0, 128], [1, 1]])
                    nc.gpsimd.dma_start(out=npad_t[:], in_=np_src)
                    nc.gpsimd.dma_start(out=v_sb[:],
                                        in_=v_d[:, :].rearrange("(n p) h -> p n h", p=128))

            # ---- phase B: attention (SW-pipelined, depth 4) ---------------------
            eT_sb = persist.tile([128, nt, nt, 128], BF16)
            S_of = {}

            def qk(t):
                Ss = [ps.tile([128, 512], F32, tag="u", bufs=8, name=f"S{j}")
                      for j in range(ng)]
                S_of[t] = Ss
                qT = qTq[t // 4]
                # group-outer so each score bank finishes (and its exp can
                # start) while the next group's matmuls still stream
                t0 = 0
                for j, g in enumerate(groups):
                    for c in range(HC):
                        nc.tensor.matmul(Ss[j][:, 0:g * 128],
                                         qT[:, t % 4, c, :],
                                         kT_sb[:, t0:t0 + g, c, :],
                                         start=(c == 0), stop=(c == HC - 1))
                    t0 += g

            def attend(t):
                Ss = S_of.pop(t)
                e = ework.tile([128, pad], BF16, tag="e")
                accg = small.tile([128, ng], F32, tag="accg")
                off = 0
                for j, g in enumerate(groups):
                    w = g * 128
                    nc.scalar.activation(out=e[:, off:off + w], in_=Ss[j][:, 0:w],
                                         func=AF.Exp, bias=0.0, scale=RSQRT_H,
                                         accum_out=accg[:, j:j + 1])
                    off += w
                dsum = small.tile([128, 1], F32, tag="dsum")
                nc.vector.reduce_sum(dsum[:], accg[:], axis=AX)
                # XBAR transpose: eT[p, kt, q] = e[q, kt*128 + p]
                nc.sync.dma_start_transpose(out=eT_sb[:, t, :, :], in_=e[:])
                # denom = sum(e) - npad  (each padded key contributes e=1;
                # the reference's +0.001 term is ~3.5e-4 of the sum: dropped)
                nc.gpsimd.tensor_scalar(out=dsum[:], in0=dsum[:], scalar1=npad_t[:],
                                        scalar2=None, op0=ALU.subtract)
                r = small.tile([128, 1], F32, tag="r")
                nc.vector.reciprocal(out=r[:], in_=dsum[:])

                C = ps.tile([128, H], F32, tag="u", bufs=8, name="C")
                for kt in range(nt):
                    nc.tensor.matmul(C[:], eT_sb[:, t, kt, :], v_sb[:, kt, :],
                                     start=(kt == 0), stop=(kt == nt - 1))
                o = osbp.tile([128, H], BF16, tag="o")
                if t % 2 == 0:
                    nc.vector.tensor_scalar_mul(o[:], C[:], r[:])
                else:
                    nc.scalar.activation(out=o[:], in_=C[:], func=AF.Copy, scale=r[:])
                nc.gpsimd.dma_start(out=out_d[t * 128:(t + 1) * 128, :], in_=o[:])

            BDEPTH = 4
            for t in range(min(BDEPTH, nt)):
                qk(t)
            for t in range(nt):
                if t + BDEPTH < nt:
                    qk(t + BDEPTH)
                attend(t)

    nc.compile()
    return nc


def _get_nc(pad, biasq, biask, affq, affk):
    key = (pad, biasq, biask, affq, affk)
    if key not in _cache:
        _cache[key] = _build(*key)
    return _cache[key]


def kernel(query, key_in, value, query_mask, key_mask,
           Wq, bq, gq, betaq, Wk, bk, gk, betak):
    query = np.asarray(query, np.float32)
    key_in = np.asarray(key_in, np.float32)
    value = np.asarray(value, np.float32)
    query_mask = np.asarray(query_mask, bool)
    key_mask = np.asarray(key_mask, bool)
    Wq = np.asarray(Wq, np.float32); Wk = np.asarray(Wk, np.float32)
    bq = np.asarray(bq, np.float32); bk = np.asarray(bk, np.float32)
    gq = np.asarray(gq, np.float32); gk = np.asarray(gk, np.float32)
    betaq = np.asarray(betaq, np.float32); betak = np.asarray(betak, np.float32)

    Q, B, Hh = query.shape
    assert Hh == H and B == NCORES

    qidx = [np.nonzero(query_mask[:, b])[0] for b in range(B)]
    kidx = [np.nonzero(key_mask[:, b])[0] for b in range(B)]
    maxn = max([len(i) for i in qidx + kidx] + [1])
    pad = max(1152, -(-maxn // 128) * 128)

    biasq = bool(np.any(bq)); biask = bool(np.any(bk))
    affq = not (np.all(gq == 1.0) and not np.any(betaq))
    affk = not (np.all(gk == 1.0) and not np.any(betak))
    nc = _get_nc(pad, biasq, biask, affq, affk)

    wqT = np.ascontiguousarray(Wq.T).astype(ml_dtypes.bfloat16)
    wkT = np.ascontiguousarray(Wk.T).astype(ml_dtypes.bfloat16)
    in_maps = []
    for b in range(B):
        qi, ki = qidx[b], kidx[b]
        xq = np.zeros((pad, H), ml_dtypes.bfloat16)
        xq[:len(qi)] = query[qi, b].astype(ml_dtypes.bfloat16)
        xk = np.zeros((pad, H), ml_dtypes.bfloat16)
        xk[:len(ki)] = key_in[ki, b].astype(ml_dtypes.bfloat16)
        vv = np.zeros((pad, H), ml_dtypes.bfloat16)
        vv[:len(ki)] = value[ki, b].astype(ml_dtypes.bfloat16)
        nt = pad // 128
        # tile-major layout [nt, 128(p), HC, 128(u)]: per-tile DMA reads are
        # 1KB-contiguous per partition
        xqt = np.ascontiguousarray(xq.reshape(nt, 128, H // 128, 128).transpose(0, 3, 2, 1))
        xkt = np.ascontiguousarray(xk.reshape(nt, 128, H // 128, 128).transpose(0, 3, 2, 1))
        m = {
            "xqT": xqt,
            "xkT": xkt,
            "v": vv,
            "npad": np.full((1, 1), pad - len(ki), np.float32),
            "WqT": wqT,
            "WkT": wkT,
        }
        if biask or affk:
            km01 = np.zeros((pad, 1), np.float32); km01[:len(ki)] = 1.0
            m["km01"] = km01
        if biasq: m["bq"] = bq.reshape(1, H)
        if biask: m["bk"] = bk.reshape(1, H)
        if affq: m["gq"] = gq.reshape(1, H); m["betaq"] = betaq.reshape(1, H)
        if affk: m["gk"] = gk.reshape(1, H); m["betak"] = betak.reshape(1, H)
        in_maps.append(m)

    res = run_bass_kernel_spmd(nc, in_maps, core_ids=list(range(NCORES)))
    global last_results
    last_results = res

    out = np.zeros((Q, B, H), np.float32)
    for b in range(B):
        qi = qidx[b]
        out[qi, b, :] = res.results[b]["out"][:len(qi)]
    return out


# revision 18
# speedup vs baseline: 1.1166x; 1.1166x over previous
"""Masked attention kernel for Trainium2, data-parallel over 8 NeuronCores.

Problem: out[q,b,:] = softmax-ish(LN(query Wq^T+bq) @ LN(key Wk^T+bk)^T / sqrt(H),
masked by query_mask & key_mask, with the reference's idiosyncratic
exp(s - 2*rowmax) / (sum + 0.001) normalization) @ value.

Key observations exploited:
 - The reference fills masked scores with the GLOBAL min before the row max.
   Every unmasked score >= global min, so the row max equals the max over
   unmasked entries whenever one exists; fully-masked rows output exactly 0.
   Hence zero cross-batch communication: B=8 batches map 1:1 onto 8 cores.
 - Masked-out query rows produce zero output rows; masked-out keys contribute
   nothing.  Both masks are ~50% dense, so each core computes attention only
   over compacted (host-gathered) rows, padded to a fixed size.
 - exp(s - 2m)/(sum + 0.001) == exp(s)/(sum' + 0.001*exp(2m)), and
   exp(2m) == (rowmax of exp(s))^2.  So we take exp with no shift at all
   (scaled scores are O(5), safely in range) and correct the denominator with
   0.001*emax^2 - npad (each padded key column contributes exactly exp(0)=1).
   This kills the entire PSUM row-max reduction pass.
 - All layout transposes (projection -> [h,seq] operands, exp(S) -> [k,q]
   stationaries for the PV matmul) run on the DMA engines' XBAR transpose
   path instead of the PE array, leaving the PE a pure matmul stream.
 - One PSUM pool with per-tag slots: 5 rotating projection/PV-accumulator
   banks plus one bank per score chunk, so phase B starts with zero
   pool-scope WAR against phase A and per-chunk exps release score banks
   incrementally.

Host side: compact/pad/transpose per batch (cheap numpy), run the SPMD NEFF,
scatter results back into the full [Q,B,H] output.
"""

import numpy as np
import ml_dtypes

import concourse.bacc as bacc
import concourse.bass as bass
import concourse.tile as tile
from concourse import mybir
from concourse.bass_utils import run_bass_kernel_spmd


def _ensure_axon_hooks():
    """concourse's trace path imports antenv.axon_hooks, which is absent in
    some containers; provide a no-op stand-in so BASS_TRACE=1 degrades to
    untraced execution instead of crashing."""
    try:
        import antenv.axon_hooks  # noqa: F401
    except ImportError:
        import sys as _sys
        import types as _types
        m = _types.ModuleType("antenv.axon_hooks")
        m._h = None
        m.set_axon_ntff_profile_hook = lambda h: setattr(m, "_h", h)
        m.get_axon_ntff_profile_hook = lambda: m._h
        _sys.modules["antenv.axon_hooks"] = m


_ensure_axon_hooks()

F32 = mybir.dt.float32
BF16 = mybir.dt.bfloat16
AX = mybir.AxisListType.X
AF = mybir.ActivationFunctionType
ALU = mybir.AluOpType

H = 512
HC = H // 128          # contraction chunks over the hidden dim
NCORES = 8
RSQRT_H = 1.0 / float(np.sqrt(np.float32(H)))
EPS = 1e-5

_cache = {}
last_results = None


def _tile_groups(nt):
    """Partition nt 128-wide key tiles into the fewest PSUM-bank groups of
    <=4 tiles (one matmul chunk / PSUM bank per group)."""
    ng = -(-nt // 4)
    base, rem = divmod(nt, ng)
    return [base + (1 if i < rem else 0) for i in range(ng)]


def _build(pad, biasq, biask, affq, affk):
    nt = pad // 128
    groups = _tile_groups(nt)
    ng = len(groups)
    nquads = -(-nt // 4)

    nc = bacc.Bacc(None, target_bir_lowering=False, debug=False, enable_asserts=False,
                   enable_partition_id=False)

    xqT_d = nc.declare_dram_parameter("xqT", [nt, 128, HC, 128], BF16, isOutput=False)
    xkT_d = nc.declare_dram_parameter("xkT", [nt, 128, HC, 128], BF16, isOutput=False)
    v_d = nc.declare_dram_parameter("v", [pad, H], BF16, isOutput=False)
    npad_d = nc.declare_dram_parameter("npad", [1, 1], F32, isOutput=False)
    km_d = None
    if biask or affk:
        km_d = nc.declare_dram_parameter("km01", [pad, 1], F32, isOutput=False)
    wqT_d = nc.declare_dram_parameter("WqT", [H, H], BF16, isOutput=False)
    wkT_d = nc.declare_dram_parameter("WkT", [H, H], BF16, isOutput=False)
    extras_d = {}
    if biasq:
        extras_d["bq"] = nc.declare_dram_parameter("bq", [1, H], F32, isOutput=False)
    if biask:
        extras_d["bk"] = nc.declare_dram_parameter("bk", [1, H], F32, isOutput=False)
    if affq:
        extras_d["gq"] = nc.declare_dram_parameter("gq", [1, H], F32, isOutput=False)
        extras_d["betaq"] = nc.declare_dram_parameter("betaq", [1, H], F32, isOutput=False)
    if affk:
        extras_d["gk"] = nc.declare_dram_parameter("gk", [1, H], F32, isOutput=False)
        extras_d["betak"] = nc.declare_dram_parameter("betak", [1, H], F32, isOutput=False)
    out_d = nc.declare_dram_parameter("out", [pad, H], BF16, isOutput=True)

    with tile.TileContext(nc) as tc:
        with (
            tc.tile_pool(name="persist", bufs=1) as persist,
            tc.tile_pool(name="small", bufs=10) as small,
            tc.tile_pool(name="lnt", bufs=4) as lnt,
            tc.tile_pool(name="ework", bufs=4) as ework,
            tc.tile_pool(name="osb", bufs=3) as osbp,
            tc.tile_pool(name="ps", bufs=1, space="PSUM") as ps,
        ):
            eps_t = persist.tile([128, 1], F32)
            nc.vector.memset(eps_t[:], EPS)

            # K-side inputs on the sync queue (its descriptors drain first),
            # Q-side on scalar; phase A runs all K jobs before Q jobs so the
            # last K transpose hides under the Q projections.  Inputs and the
            # transposed Q operand live in per-pair tiles so consumers wait on
            # exactly one DMA (tile-granular dependency tracking).
            wk_sb = persist.tile([128, HC, H], BF16)
            wq_sb = persist.tile([128, HC, H], BF16)
            wk_r = wkT_d[:, :].rearrange("(c p) i -> p c i", p=128)
            wq_r = wqT_d[:, :].rearrange("(c p) i -> p c i", p=128)
            xkp = [persist.tile([128, 2, HC, 128], BF16, tag=f"xkp{j}", name=f"xkp{j}")
                   for j in range(-(-nt // 2))]
            xqp = [persist.tile([128, 2, HC, 128], BF16, tag=f"xqp{j}", name=f"xqp{j}")
                   for j in range(-(-nt // 2))]
            nc.sync.dma_start(out=wq_sb[:], in_=wq_r[:])
            for j in range(2):
                t0, t1 = 2 * j, min(2 * j + 2, nt)
                nc.sync.dma_start(out=xqp[j][:, 0:t1 - t0, :, :],
                                  in_=xqT_d[t0:t1, :, :, :].rearrange("t p c u -> p t c u"))
            nc.scalar.dma_start(out=wk_sb[:], in_=wk_r[:])
            for j in range(-(-nt // 2)):
                t0, t1 = 2 * j, min(2 * j + 2, nt)
                nc.scalar.dma_start(out=xkp[j][:, 0:t1 - t0, :, :],
                                    in_=xkT_d[t0:t1, :, :, :].rearrange("t p c u -> p t c u"))
            for j in range(2, -(-nt // 2)):
                t0, t1 = 2 * j, min(2 * j + 2, nt)
                nc.scalar.dma_start(out=xqp[j][:, 0:t1 - t0, :, :],
                                    in_=xqT_d[t0:t1, :, :, :].rearrange("t p c u -> p t c u"))
            npad_t = persist.tile([128, 1], F32)
            kmA_sb = None
            if km_d is not None:
                kmA_sb = persist.tile([128, nt], F32)
                nc.gpsimd.dma_start(out=kmA_sb[:], in_=km_d[:, :].rearrange("(n p) o -> p (n o)", p=128))
            bcast = {}
            for name, dram in extras_d.items():
                tb = persist.tile([128, H], F32, tag=f"bc_{name}")
                src = dram[:, :]
                src = bass.AP(tensor=src.tensor, offset=src.offset, ap=[[0, 128]] + [src.ap[-1]])
                nc.gpsimd.dma_start(out=tb[:], in_=src)
                bcast[name] = tb

            # [128 h_sub, tile, h_chunk, 128 seq] operand layouts for attention.
            # kT is one tile (QK^T moving operand spans key tiles; every use
            # needs all of it anyway); qT is per-pair (each QK^T stationary
            # needs exactly one pair's XBAR transpose).
            qTq = [persist.tile([128, 4, HC, 128], BF16, tag=f"qTq{j}", name=f"qTq{j}")
                   for j in range(nquads)]
            kT_sb = persist.tile([128, nt, HC, 128], BF16)
            v_sb = persist.tile([128, nt, H], BF16)

            # ---- phase A: project + layernorm (XBAR-transposed evacuation) ------
            nq0 = min(4, nt)
            jobs = ([(0, t) for t in range(nq0)]
                    + [(1, t) for t in range(nt)]
                    + [(0, t) for t in range(nq0, nt)])
            ps_of = {}
            lnp_of = {}

            def proj(i):
                s, t = jobs[i]
                x_sb = (xqp, xkp)[s][t // 2][:, t % 2, :, :]
                w_sb = (wq_sb, wk_sb)[s]
                p = ps.tile([128, H], F32, tag="u", bufs=8)
                ps_of[i] = p
                for c in range(HC):
                    nc.tensor.matmul(p[:], x_sb[:, c, :],
                                     w_sb[:, c, :], start=(c == 0), stop=(c == HC - 1))

            def ln_and_tp(i):
                s, t = jobs[i]
                use_bias = (biasq, biask)[s]
                use_aff = (affq, affk)[s]
                p = ps_of.pop(i)
                if use_bias:
                    nc.vector.tensor_add(p[:], p[:], bcast[("bq", "bk")[s]][:])
                stats = small.tile([128, 6], F32, tag="stats")
                nc.vector.bn_stats(out=stats[:], in_=p[:])
                mv = small.tile([128, 2], F32, tag="mv")
                nc.vector.bn_aggr(out=mv[:], in_=stats[:])
                sd = small.tile([128, 1], F32, tag="sd")
                nc.scalar.activation(out=sd[:], in_=mv[:, 1:2], func=AF.Sqrt,
                                     bias=eps_t[:], scale=1.0)
                rstd = small.tile([128, 1], F32, tag="rstd")
                nc.vector.reciprocal(out=rstd[:], in_=sd[:])
                nmr = small.tile([128, 1], F32, tag="nmr")
                nc.gpsimd.tensor_mul(nmr[:], mv[:, 0:1], rstd[:])
                nc.gpsimd.tensor_scalar_mul(nmr[:], nmr[:], -1.0)
                # batch up to four tiles of one side into a single XBAR
                # transpose (amortizes the ~1.3us per-transpose engine cost)
                pid, slot = t // 4, t % 4
                if slot == 0:
                    lnp = lnt.tile([128, 4, H], BF16, tag="lnp")
                    lnp_of[(s, pid)] = lnp
                else:
                    lnp = lnp_of[(s, pid)]
                ln = lnp[:, slot, :]
                if use_aff:
                    ln32 = lnt.tile([128, H], F32, tag="ln32")
                    nc.scalar.activation(out=ln32[:], in_=p[:], func=AF.Identity,
                                         bias=nmr[:], scale=rstd[:])
                    nc.vector.tensor_mul(ln32[:], ln32[:], bcast[("gq", "gk")[s]][:])
                    nc.vector.tensor_add(ln, ln32[:], bcast[("betaq", "betak")[s]][:])
                else:
                    nc.scalar.activation(out=ln, in_=p[:], func=AF.Identity,
                                         bias=nmr[:], scale=rstd[:])
                if s == 1 and kmA_sb is not None:
                    nc.vector.tensor_scalar_mul(ln, ln, kmA_sb[:, t:t + 1])
                # XBAR transpose on sync only: dst[p, c, q] = ln[q, c*128+p]
                if slot == 3 or t == nt - 1:
                    if s == 1:
                        dst = kT_sb[:, 4 * pid:4 * pid + slot + 1, :, :]
                    else:
                        dst = qTq[pid][:, 0:slot + 1, :, :]
                    if slot == 3:
                        nc.sync.dma_start_transpose(out=dst, in_=lnp[:])
                    else:
                        nc.sync.dma_start_transpose(out=dst, in_=lnp[:, 0:slot + 1, :])
                    lnp_of.pop((s, pid))

            DEPTH = 7
            for i in range(min(DEPTH, len(jobs))):
                proj(i)
            for i in range(len(jobs)):
                if i + DEPTH < len(jobs):
                    proj(i + DEPTH)
                ln_and_tp(i)
                if i == 6:
                    # defer bulk/broadcast loads so their descriptors don't
                    # starve the phase-A input stream
                    np_src = npad_d[:, :]
                    np_src = bass.AP(tensor=np_src.tensor, offset=np_src.offset,
                                     ap=[[0, 128], [1, 1]])
                    nc.gpsimd.dma_start(out=npad_t[:], in_=np_src)
                    nc.gpsimd.dma_start(out=v_sb[:],
                                        in_=v_d[:, :].rearrange("(n p) h -> p n h", p=128))

            # ---- phase B: attention (SW-pipelined, depth 4) ---------------------
            eT_sb = persist.tile([128, nt, nt, 128], BF16)
            S_of = {}

            def qk(t):
                Ss = [ps.tile([128, 512], F32, tag="u", bufs=8, name=f"S{j}")
                      for j in range(ng)]
                S_of[t] = Ss
                qT = qTq[t // 4]
                # group-outer so each score bank finishes (and its exp can
                # start) while the next group's matmuls still stream
                t0 = 0
                for j, g in enumerate(groups):
                    for c in range(HC):
                        nc.tensor.matmul(Ss[j][:, 0:g * 128],
                                         qT[:, t % 4, c, :],
                                         kT_sb[:, t0:t0 + g, c, :],
                                         start=(c == 0), stop=(c == HC - 1))
                    t0 += g

            def attend(t):
                Ss = S_of.pop(t)
                e = ework.tile([128, pad], BF16, tag="e")
                accg = small.tile([128, ng], F32, tag="accg")
                off = 0
                for j, g in enumerate(groups):
                    w = g * 128
                    nc.scalar.activation(out=e[:, off:off + w], in_=Ss[j][:, 0:w],
                                         func=AF.Exp, bias=0.0, scale=RSQRT_H,
                                         accum_out=accg[:, j:j + 1])
                    off += w
                dsum = small.tile([128, 1], F32, tag="dsum")
                nc.vector.reduce_sum(dsum[:], accg[:], axis=AX)
                # XBAR transpose: eT[p, kt, q] = e[q, kt*128 + p]
                nc.sync.dma_start_transpose(out=eT_sb[:, t, :, :], in_=e[:])
                # denom = sum(e) - npad  (each padded key contributes e=1;
                # the reference's +0.001 term is ~3.5e-4 of the sum: dropped)
                nc.gpsimd.tensor_scalar(out=dsum[:], in0=dsum[:], scalar1=npad_t[:],
                                        scalar2=None, op0=ALU.subtract)
                r = small.tile([128, 1], F32, tag="r")
                nc.vector.reciprocal(out=r[:], in_=dsum[:])

                C = ps.tile([128, H], F32, tag="u", bufs=8, name="C")
                for kt in range(nt):
                    nc.tensor.matmul(C[:], eT_sb[:, t, kt, :], v_sb[:, kt, :],
                                     start=(kt == 0), stop=(kt == nt - 1))
                o = osbp.tile([128, H], BF16, tag="o")
                if t % 2 == 0:
                    nc.vector.tensor_scalar_mul(o[:], C[:], r[:])
                else:
                    nc.scalar.activation(out=o[:], in_=C[:], func=AF.Copy, scale=r[:])
                oeng = (nc.sync, nc.scalar)[t % 2]
                oeng.dma_start(out=out_d[t * 128:(t + 1) * 128, :], in_=o[:])

            BDEPTH = 3
            for t in range(min(BDEPTH, nt)):
                qk(t)
            for t in range(nt):
                if t + BDEPTH < nt:
                    qk(t + BDEPTH)
                attend(t)

    nc.compile()
    return nc


def _get_nc(pad, biasq, biask, affq, affk):
    key = (pad, biasq, biask, affq, affk)
    if key not in _cache:
        _cache[key] = _build(*key)
    return _cache[key]


def kernel(query, key_in, value, query_mask, key_mask,
           Wq, bq, gq, betaq, Wk, bk, gk, betak):
    query = np.asarray(query, np.float32)
    key_in = np.asarray(key_in, np.float32)
    value = np.asarray(value, np.float32)
    query_mask = np.asarray(query_mask, bool)
    key_mask = np.asarray(key_mask, bool)
    Wq = np.asarray(Wq, np.float32); Wk = np.asarray(Wk, np.float32)
    bq = np.asarray(bq, np.float32); bk = np.asarray(bk, np.float32)
    gq = np.asarray(gq, np.float32); gk = np.asarray(gk, np.float32)
    betaq = np.asarray(betaq, np.float32); betak = np.asarray(betak, np.float32)

    Q, B, Hh = query.shape
    assert Hh == H and B == NCORES

    qidx = [np.nonzero(query_mask[:, b])[0] for b in range(B)]
    kidx = [np.nonzero(key_mask[:, b])[0] for b in range(B)]
    maxn = max([len(i) for i in qidx + kidx] + [1])
    pad = max(1152, -(-maxn // 128) * 128)

    biasq = bool(np.any(bq)); biask = bool(np.any(bk))
    affq = not (np.all(gq == 1.0) and not np.any(betaq))
    affk = not (np.all(gk == 1.0) and not np.any(betak))
    nc = _get_nc(pad, biasq, biask, affq, affk)

    wqT = np.ascontiguousarray(Wq.T).astype(ml_dtypes.bfloat16)
    wkT = np.ascontiguousarray(Wk.T).astype(ml_dtypes.bfloat16)
    in_maps = []
    for b in range(B):
        qi, ki = qidx[b], kidx[b]
        xq = np.zeros((pad, H), ml_dtypes.bfloat16)
        xq[:len(qi)] = query[qi, b].astype(ml_dtypes.bfloat16)
        xk = np.zeros((pad, H), ml_dtypes.bfloat16)
        xk[:len(ki)] = key_in[ki, b].astype(ml_dtypes.bfloat16)
        vv = np.zeros((pad, H), ml_dtypes.bfloat16)
        vv[:len(ki)] = value[ki, b].astype(ml_dtypes.bfloat16)
        nt = pad // 128
        # tile-major layout [nt, 128(p), HC, 128(u)]: per-tile DMA reads are
        # 1KB-contiguous per partition
        xqt = np.ascontiguousarray(xq.reshape(nt, 128, H // 128, 128).transpose(0, 3, 2, 1))
        xkt = np.ascontiguousarray(xk.reshape(nt, 128, H // 128, 128).transpose(0, 3, 2, 1))
        m = {
            "xqT": xqt,
            "xkT": xkt,
            "v": vv,
            "npad": np.full((1, 1), pad - len(ki), np.float32),
            "WqT": wqT,
            "WkT": wkT,
        }
        if biask or affk:
            km01 = np.zeros((pad, 1), np.float32); km01[:len(ki)] = 1.0
            m["km01"] = km01
        if biasq: m["bq"] = bq.reshape(1, H)
        if biask: m["bk"] = bk.reshape(1, H)
        if affq: m["gq"] = gq.reshape(1, H); m["betaq"] = betaq.reshape(1, H)
        if affk: m["gk"] = gk.reshape(1, H); m["betak"] = betak.reshape(1, H)
        in_maps.append(m)

    res = run_bass_kernel_spmd(nc, in_maps, core_ids=list(range(NCORES)))
    global last_results
    last_results = res

    out = np.zeros((Q, B, H), np.float32)
    for b in range(B):
        qi = qidx[b]
        out[qi, b, :] = res.results[b]["out"][:len(qi)]
    return out


# revision 19
# speedup vs baseline: 1.1189x; 1.0021x over previous
"""Masked attention kernel for Trainium2, data-parallel over 8 NeuronCores.

Problem: out[q,b,:] = softmax-ish(LN(query Wq^T+bq) @ LN(key Wk^T+bk)^T / sqrt(H),
masked by query_mask & key_mask, with the reference's idiosyncratic
exp(s - 2*rowmax) / (sum + 0.001) normalization) @ value.

Key observations exploited:
 - The reference fills masked scores with the GLOBAL min before the row max.
   Every unmasked score >= global min, so the row max equals the max over
   unmasked entries whenever one exists; fully-masked rows output exactly 0.
   Hence zero cross-batch communication: B=8 batches map 1:1 onto 8 cores.
 - Masked-out query rows produce zero output rows; masked-out keys contribute
   nothing.  Both masks are ~50% dense, so each core computes attention only
   over compacted (host-gathered) rows, padded to a fixed size.
 - exp(s - 2m)/(sum + 0.001) == exp(s)/(sum' + 0.001*exp(2m)), and
   exp(2m) == (rowmax of exp(s))^2.  So we take exp with no shift at all
   (scaled scores are O(5), safely in range) and correct the denominator with
   0.001*emax^2 - npad (each padded key column contributes exactly exp(0)=1).
   This kills the entire PSUM row-max reduction pass.
 - All layout transposes (projection -> [h,seq] operands, exp(S) -> [k,q]
   stationaries for the PV matmul) run on the DMA engines' XBAR transpose
   path instead of the PE array, leaving the PE a pure matmul stream.
 - One PSUM pool with per-tag slots: 5 rotating projection/PV-accumulator
   banks plus one bank per score chunk, so phase B starts with zero
   pool-scope WAR against phase A and per-chunk exps release score banks
   incrementally.

Host side: compact/pad/transpose per batch (cheap numpy), run the SPMD NEFF,
scatter results back into the full [Q,B,H] output.
"""

import numpy as np
import ml_dtypes

import concourse.bacc as bacc
import concourse.bass as bass
import concourse.tile as tile
from concourse import mybir
from concourse.bass_utils import run_bass_kernel_spmd


def _ensure_axon_hooks():
    """concourse's trace path imports antenv.axon_hooks, which is absent in
    some containers; provide a no-op stand-in so BASS_TRACE=1 degrades to
    untraced execution instead of crashing."""
    try:
        import antenv.axon_hooks  # noqa: F401
    except ImportError:
        import sys as _sys
        import types as _types
        m = _types.ModuleType("antenv.axon_hooks")
        m._h = None
        m.set_axon_ntff_profile_hook = lambda h: setattr(m, "_h", h)
        m.get_axon_ntff_profile_hook = lambda: m._h
        _sys.modules["antenv.axon_hooks"] = m


_ensure_axon_hooks()

F32 = mybir.dt.float32
BF16 = mybir.dt.bfloat16
AX = mybir.AxisListType.X
AF = mybir.ActivationFunctionType
ALU = mybir.AluOpType

H = 512
HC = H // 128          # contraction chunks over the hidden dim
NCORES = 8
RSQRT_H = 1.0 / float(np.sqrt(np.float32(H)))
EPS = 1e-5

_cache = {}
last_results = None


def _tile_groups(nt):
    """Partition nt 128-wide key tiles into the fewest PSUM-bank groups of
    <=4 tiles (one matmul chunk / PSUM bank per group)."""
    ng = -(-nt // 4)
    base, rem = divmod(nt, ng)
    return [base + (1 if i < rem else 0) for i in range(ng)]


def _build(pad, biasq, biask, affq, affk):
    nt = pad // 128
    groups = _tile_groups(nt)
    ng = len(groups)
    kbat = [4] * (nt // 4) + ([nt % 4] if nt % 4 else [])
    qbat = [2, 2] if nt >= 4 else [min(2, nt)] + ([2] if nt > 2 else [])
    rem = nt - sum(qbat)
    qbat += [4] * (rem // 4) + ([rem % 4] if rem % 4 else [])
    def _bmap(bat):
        m = {}
        base = 0
        for i, b in enumerate(bat):
            for s2 in range(b):
                m[base + s2] = (i, s2, base, b)
            base += b
        return m
    kmap, qmap = _bmap(kbat), _bmap(qbat)

    nc = bacc.Bacc(None, target_bir_lowering=False, debug=False, enable_asserts=False,
                   enable_partition_id=False)

    xqT_d = nc.declare_dram_parameter("xqT", [nt, 128, HC, 128], BF16, isOutput=False)
    xkT_d = nc.declare_dram_parameter("xkT", [nt, 128, HC, 128], BF16, isOutput=False)
    v_d = nc.declare_dram_parameter("v", [pad, H], BF16, isOutput=False)
    npad_d = nc.declare_dram_parameter("npad", [1, 1], F32, isOutput=False)
    km_d = None
    if biask or affk:
        km_d = nc.declare_dram_parameter("km01", [pad, 1], F32, isOutput=False)
    wqT_d = nc.declare_dram_parameter("WqT", [H, H], BF16, isOutput=False)
    wkT_d = nc.declare_dram_parameter("WkT", [H, H], BF16, isOutput=False)
    extras_d = {}
    if biasq:
        extras_d["bq"] = nc.declare_dram_parameter("bq", [1, H], F32, isOutput=False)
    if biask:
        extras_d["bk"] = nc.declare_dram_parameter("bk", [1, H], F32, isOutput=False)
    if affq:
        extras_d["gq"] = nc.declare_dram_parameter("gq", [1, H], F32, isOutput=False)
        extras_d["betaq"] = nc.declare_dram_parameter("betaq", [1, H], F32, isOutput=False)
    if affk:
        extras_d["gk"] = nc.declare_dram_parameter("gk", [1, H], F32, isOutput=False)
        extras_d["betak"] = nc.declare_dram_parameter("betak", [1, H], F32, isOutput=False)
    out_d = nc.declare_dram_parameter("out", [pad, H], BF16, isOutput=True)

    with tile.TileContext(nc) as tc:
        with (
            tc.tile_pool(name="persist", bufs=1) as persist,
            tc.tile_pool(name="small", bufs=10) as small,
            tc.tile_pool(name="lnt", bufs=4) as lnt,
            tc.tile_pool(name="ework", bufs=4) as ework,
            tc.tile_pool(name="osb", bufs=3) as osbp,
            tc.tile_pool(name="ps", bufs=1, space="PSUM") as ps,
        ):
            eps_t = persist.tile([128, 1], F32)
            nc.vector.memset(eps_t[:], EPS)

            # K-side inputs on the sync queue (its descriptors drain first),
            # Q-side on scalar; phase A runs all K jobs before Q jobs so the
            # last K transpose hides under the Q projections.  Inputs and the
            # transposed Q operand live in per-pair tiles so consumers wait on
            # exactly one DMA (tile-granular dependency tracking).
            wk_sb = persist.tile([128, HC, H], BF16)
            wq_sb = persist.tile([128, HC, H], BF16)
            wk_r = wkT_d[:, :].rearrange("(c p) i -> p c i", p=128)
            wq_r = wqT_d[:, :].rearrange("(c p) i -> p c i", p=128)
            xkp = [persist.tile([128, 2, HC, 128], BF16, tag=f"xkp{j}", name=f"xkp{j}")
                   for j in range(-(-nt // 2))]
            xqp = [persist.tile([128, 2, HC, 128], BF16, tag=f"xqp{j}", name=f"xqp{j}")
                   for j in range(-(-nt // 2))]
            nc.sync.dma_start(out=wq_sb[:], in_=wq_r[:])
            for j in range(2):
                t0, t1 = 2 * j, min(2 * j + 2, nt)
                nc.sync.dma_start(out=xqp[j][:, 0:t1 - t0, :, :],
                                  in_=xqT_d[t0:t1, :, :, :].rearrange("t p c u -> p t c u"))
            nc.scalar.dma_start(out=wk_sb[:], in_=wk_r[:])
            for j in range(-(-nt // 2)):
                t0, t1 = 2 * j, min(2 * j + 2, nt)
                nc.scalar.dma_start(out=xkp[j][:, 0:t1 - t0, :, :],
                                    in_=xkT_d[t0:t1, :, :, :].rearrange("t p c u -> p t c u"))
            for j in range(2, -(-nt // 2)):
                t0, t1 = 2 * j, min(2 * j + 2, nt)
                nc.scalar.dma_start(out=xqp[j][:, 0:t1 - t0, :, :],
                                    in_=xqT_d[t0:t1, :, :, :].rearrange("t p c u -> p t c u"))
            npad_t = persist.tile([128, 1], F32)
            kmA_sb = None
            if km_d is not None:
                kmA_sb = persist.tile([128, nt], F32)
                nc.gpsimd.dma_start(out=kmA_sb[:], in_=km_d[:, :].rearrange("(n p) o -> p (n o)", p=128))
            bcast = {}
            for name, dram in extras_d.items():
                tb = persist.tile([128, H], F32, tag=f"bc_{name}")
                src = dram[:, :]
                src = bass.AP(tensor=src.tensor, offset=src.offset, ap=[[0, 128]] + [src.ap[-1]])
                nc.gpsimd.dma_start(out=tb[:], in_=src)
                bcast[name] = tb

            # [128 h_sub, tile, h_chunk, 128 seq] operand layouts for attention.
            # kT is one tile (QK^T moving operand spans key tiles; every use
            # needs all of it anyway); qT is per-pair (each QK^T stationary
            # needs exactly one pair's XBAR transpose).
            qTb = [persist.tile([128, qbat[j], HC, 128], BF16, tag=f"qTb{j}", name=f"qTb{j}")
                   for j in range(len(qbat))]
            kT_sb = persist.tile([128, nt, HC, 128], BF16)
            v_sb = persist.tile([128, nt, H], BF16)

            # ---- phase A: project + layernorm (XBAR-transposed evacuation) ------
            nq0 = min(2, nt)
            jobs = ([(0, t) for t in range(nq0)]
                    + [(1, t) for t in range(nt)]
                    + [(0, t) for t in range(nq0, nt)])
            ps_of = {}
            lnp_of = {}

            def proj(i):
                s, t = jobs[i]
                x_sb = (xqp, xkp)[s][t // 2][:, t % 2, :, :]
                w_sb = (wq_sb, wk_sb)[s]
                p = ps.tile([128, H], F32, tag="u", bufs=8)
                ps_of[i] = p
                for c in range(HC):
                    nc.tensor.matmul(p[:], x_sb[:, c, :],
                                     w_sb[:, c, :], start=(c == 0), stop=(c == HC - 1))

            def ln_and_tp(i):
                s, t = jobs[i]
                use_bias = (biasq, biask)[s]
                use_aff = (affq, affk)[s]
                p = ps_of.pop(i)
                if use_bias:
                    nc.vector.tensor_add(p[:], p[:], bcast[("bq", "bk")[s]][:])
                stats = small.tile([128, 6], F32, tag="stats")
                nc.vector.bn_stats(out=stats[:], in_=p[:])
                mv = small.tile([128, 2], F32, tag="mv")
                nc.vector.bn_aggr(out=mv[:], in_=stats[:])
                sd = small.tile([128, 1], F32, tag="sd")
                nc.scalar.activation(out=sd[:], in_=mv[:, 1:2], func=AF.Sqrt,
                                     bias=eps_t[:], scale=1.0)
                rstd = small.tile([128, 1], F32, tag="rstd")
                nc.vector.reciprocal(out=rstd[:], in_=sd[:])
                nmr = small.tile([128, 1], F32, tag="nmr")
                nc.gpsimd.tensor_mul(nmr[:], mv[:, 0:1], rstd[:])
                nc.gpsimd.tensor_scalar_mul(nmr[:], nmr[:], -1.0)
                # batch tiles of one side into a single XBAR transpose
                # (amortizes the ~1.3us per-transpose engine cost)
                pid, slot, base, bsz = (qmap, kmap)[s][t]
                if slot == 0:
                    lnp = lnt.tile([128, 4, H], BF16, tag="lnp")
                    lnp_of[(s, pid)] = lnp
                else:
                    lnp = lnp_of[(s, pid)]
                ln = lnp[:, slot, :]
                if use_aff:
                    ln32 = lnt.tile([128, H], F32, tag="ln32")
                    nc.scalar.activation(out=ln32[:], in_=p[:], func=AF.Identity,
                                         bias=nmr[:], scale=rstd[:])
                    nc.vector.tensor_mul(ln32[:], ln32[:], bcast[("gq", "gk")[s]][:])
                    nc.vector.tensor_add(ln, ln32[:], bcast[("betaq", "betak")[s]][:])
                elif s == 0 and t >= 4:
                    # late Q jobs: apply on the vector engine so phase B's
                    # first exps aren't queued behind these on scalar
                    nc.vector.tensor_scalar(out=ln, in0=p[:], scalar1=rstd[:],
                                            scalar2=nmr[:], op0=ALU.mult, op1=ALU.add)
                else:
                    nc.scalar.activation(out=ln, in_=p[:], func=AF.Identity,
                                         bias=nmr[:], scale=rstd[:])
                if s == 1 and kmA_sb is not None:
                    nc.vector.tensor_scalar_mul(ln, ln, kmA_sb[:, t:t + 1])
                # XBAR transpose on sync only: dst[p, c, q] = ln[q, c*128+p]
                if slot == bsz - 1:
                    if s == 1:
                        dst = kT_sb[:, base:base + bsz, :, :]
                    else:
                        dst = qTb[pid][:, 0:bsz, :, :]
                    nc.sync.dma_start_transpose(out=dst, in_=lnp[:, 0:bsz, :])
                    lnp_of.pop((s, pid))

            DEPTH = 7
            for i in range(min(DEPTH, len(jobs))):
                proj(i)
            for i in range(len(jobs)):
                if i + DEPTH < len(jobs):
                    proj(i + DEPTH)
                ln_and_tp(i)
                if i == 6:
                    # defer bulk/broadcast loads so their descriptors don't
                    # starve the phase-A input stream
                    np_src = npad_d[:, :]
                    np_src = bass.AP(tensor=np_src.tensor, offset=np_src.offset,
                                     ap=[[0, 128], [1, 1]])
                    nc.gpsimd.dma_start(out=npad_t[:], in_=np_src)
                    nc.gpsimd.dma_start(out=v_sb[:],
                                        in_=v_d[:, :].rearrange("(n p) h -> p n h", p=128))

            # ---- phase B: attention (SW-pipelined, depth 4) ---------------------
            eT_sb = persist.tile([128, nt, nt, 128], BF16)
            S_of = {}

            def qk(t):
                Ss = [ps.tile([128, 512], F32, tag="u", bufs=8, name=f"S{j}")
                      for j in range(ng)]
                S_of[t] = Ss
                qi, qs, _, _ = qmap[t]
                qT = qTb[qi]
                # group-outer so each score bank finishes (and its exp can
                # start) while the next group's matmuls still stream
                t0 = 0
                for j, g in enumerate(groups):
                    for c in range(HC):
                        nc.tensor.matmul(Ss[j][:, 0:g * 128],
                                         qT[:, qs, c, :],
                                         kT_sb[:, t0:t0 + g, c, :],
                                         start=(c == 0), stop=(c == HC - 1))
                    t0 += g

            def attend(t):
                Ss = S_of.pop(t)
                e = ework.tile([128, pad], BF16, tag="e")
                accg = small.tile([128, ng], F32, tag="accg")
                off = 0
                for j, g in enumerate(groups):
                    w = g * 128
                    nc.scalar.activation(out=e[:, off:off + w], in_=Ss[j][:, 0:w],
                                         func=AF.Exp, bias=0.0, scale=RSQRT_H,
                                         accum_out=accg[:, j:j + 1])
                    off += w
                dsum = small.tile([128, 1], F32, tag="dsum")
                nc.vector.reduce_sum(dsum[:], accg[:], axis=AX)
                # XBAR transpose: eT[p, kt, q] = e[q, kt*128 + p]
                nc.sync.dma_start_transpose(out=eT_sb[:, t, :, :], in_=e[:])
                # denom = sum(e) - npad  (each padded key contributes e=1;
                # the reference's +0.001 term is ~3.5e-4 of the sum: dropped)
                nc.gpsimd.tensor_scalar(out=dsum[:], in0=dsum[:], scalar1=npad_t[:],
                                        scalar2=None, op0=ALU.subtract)
                r = small.tile([128, 1], F32, tag="r")
                nc.vector.reciprocal(out=r[:], in_=dsum[:])

                C = ps.tile([128, H], F32, tag="u", bufs=8, name="C")
                for kt in range(nt):
                    nc.tensor.matmul(C[:], eT_sb[:, t, kt, :], v_sb[:, kt, :],
                                     start=(kt == 0), stop=(kt == nt - 1))
                o = osbp.tile([128, H], BF16, tag="o")
                nc.vector.tensor_scalar_mul(o[:], C[:], r[:])
                oeng = (nc.sync, nc.scalar)[t % 2]
                oeng.dma_start(out=out_d[t * 128:(t + 1) * 128, :], in_=o[:])

            BDEPTH = 3
            for t in range(min(BDEPTH, nt)):
                qk(t)
            for t in range(nt):
                if t + BDEPTH < nt:
                    qk(t + BDEPTH)
                attend(t)

    nc.compile()
    return nc


def _get_nc(pad, biasq, biask, affq, affk):
    key = (pad, biasq, biask, affq, affk)
    if key not in _cache:
        _cache[key] = _build(*key)
    return _cache[key]


def kernel(query, key_in, value, query_mask, key_mask,
           Wq, bq, gq, betaq, Wk, bk, gk, betak):
    query = np.asarray(query, np.float32)
    key_in = np.asarray(key_in, np.float32)
    value = np.asarray(value, np.float32)
    query_mask = np.asarray(query_mask, bool)
    key_mask = np.asarray(key_mask, bool)
    Wq = np.asarray(Wq, np.float32); Wk = np.asarray(Wk, np.float32)
    bq = np.asarray(bq, np.float32); bk = np.asarray(bk, np.float32)
    gq = np.asarray(gq, np.float32); gk = np.asarray(gk, np.float32)
    betaq = np.asarray(betaq, np.float32); betak = np.asarray(betak, np.float32)

    Q, B, Hh = query.shape
    assert Hh == H and B == NCORES

    qidx = [np.nonzero(query_mask[:, b])[0] for b in range(B)]
    kidx = [np.nonzero(key_mask[:, b])[0] for b in range(B)]
    maxn = max([len(i) for i in qidx + kidx] + [1])
    pad = max(1152, -(-maxn // 128) * 128)

    biasq = bool(np.any(bq)); biask = bool(np.any(bk))
    affq = not (np.all(gq == 1.0) and not np.any(betaq))
    affk = not (np.all(gk == 1.0) and not np.any(betak))
    nc = _get_nc(pad, biasq, biask, affq, affk)

    wqT = np.ascontiguousarray(Wq.T).astype(ml_dtypes.bfloat16)
    wkT = np.ascontiguousarray(Wk.T).astype(ml_dtypes.bfloat16)
    in_maps = []
    for b in range(B):
        qi, ki = qidx[b], kidx[b]
        xq = np.zeros((pad, H), ml_dtypes.bfloat16)
        xq[:len(qi)] = query[qi, b].astype(ml_dtypes.bfloat16)
        xk = np.zeros((pad, H), ml_dtypes.bfloat16)
        xk[:len(ki)] = key_in[ki, b].astype(ml_dtypes.bfloat16)
        vv = np.zeros((pad, H), ml_dtypes.bfloat16)
        vv[:len(ki)] = value[ki, b].astype(ml_dtypes.bfloat16)
        nt = pad // 128
        # tile-major layout [nt, 128(p), HC, 128(u)]: per-tile DMA reads are
        # 1KB-contiguous per partition
        xqt = np.ascontiguousarray(xq.reshape(nt, 128, H // 128, 128).transpose(0, 3, 2, 1))
        xkt = np.ascontiguousarray(xk.reshape(nt, 128, H // 128, 128).transpose(0, 3, 2, 1))
        m = {
            "xqT": xqt,
            "xkT": xkt,
            "v": vv,
            "npad": np.full((1, 1), pad - len(ki), np.float32),
            "WqT": wqT,
            "WkT": wkT,
        }
        if biask or affk:
            km01 = np.zeros((pad, 1), np.float32); km01[:len(ki)] = 1.0
            m["km01"] = km01
        if biasq: m["bq"] = bq.reshape(1, H)
        if biask: m["bk"] = bk.reshape(1, H)
        if affq: m["gq"] = gq.reshape(1, H); m["betaq"] = betaq.reshape(1, H)
        if affk: m["gk"] = gk.reshape(1, H); m["betak"] = betak.reshape(1, H)
        in_maps.append(m)

    res = run_bass_kernel_spmd(nc, in_maps, core_ids=list(range(NCORES)))
    global last_results
    last_results = res

    out = np.zeros((Q, B, H), np.float32)
    for b in range(B):
        qi = qidx[b]
        out[qi, b, :] = res.results[b]["out"][:len(qi)]
    return out
